# revision 18
# baseline (speedup 1.0000x reference)
"""Multi-head attention kernel for Trainium2 (8 NeuronCores, SPMD).

Sharding: core c handles batch b=c//2 and 4 of the 8 heads
(projection columns 128*(c%2) .. +128).  Each core computes a partial
output projection (contracting only its own 128 head-dims); the host sums
the two partials per batch and adds bo.

v5: keep the PE saturated (sustained back-to-back matmuls reach ~2.3GHz
on this part; gaps drop it to ~1.2GHz):
  - lag-2 software pipeline: AV matmuls for tile t are emitted after the
    score matmuls of tile t+2, so the PE never waits on exp/mask-mul.
  - AV PSUM packed 2 heads per bank (even head rows 0-32, odd rows
    64-96 via tile_position col 64) -> 2 banks for AV, 3 double-bank
    score buffers in flight.
  - scores pre-scaled by a = 2^10/ln2 (sqrt(a) in Wq and Wk host-side);
    exp tiles use Act with scale=1/a; Schraudolph tiles compute
    bitcast_f16(int16((s*a+b)*m)) in ONE DVE op (~4%% sawtooth).
  - mask 0/1 fp16 multiplied per head with packed operands (DVE 2-byte
    fast mode) or on GpSimd for a slice of tiles.
  - normalize: reciprocal of the fused denominator row + DMA
    partition-broadcast + DVE mul (no PE ones-matmuls).
  - output projection folded into the j loop to fill PE gaps.
"""

import numpy as np
import ml_dtypes

import concourse.bass as bass
import concourse.tile as tile
from concourse import bacc, mybir
from concourse.bass_utils import run_bass_kernel_spmd
from concourse._compat import with_exitstack
from contextlib import ExitStack

B, D = 4, 256
H = 8
PROJ = 256
DH = PROJ // H            # 32
NCORES = 8
HPC = H // 2              # heads per core
PC = HPC * DH             # projection cols per core = 128
QB = 512                  # q block (PE moving dim / PSUM bank)
KBK = 128                 # k block

EXPA = float(1024.0 / np.log(2.0))   # score pre-scale (2^10 * log2 e)
SCHRB = 15360.0 - 60.0               # Schraudolph bias, c=60 (zero-mean)

import os as _os
# Schraudolph kb blocks: (kb % SCHR_DEN) < SCHR_NUM
SCHR_NUM = int(_os.environ.get("K_SCHR_NUM", "1"))
SCHR_DEN = int(_os.environ.get("K_SCHR_DEN", "4"))
# per-head mask-muls on GpSimd: (idx % GP_DEN) < GP_NUM
GP_NUM = int(_os.environ.get("K_GP_NUM", "1"))
GP_DEN = int(_os.environ.get("K_GP_DEN", "4"))
# DMA partition-broadcast for 1/den (fallback: PE ones-matmul)
BCAST_DMA = _os.environ.get("K_BCAST_DMA", "0") == "1"
LAG = int(_os.environ.get("K_LAG", "1"))

F32 = mybir.dt.float32
F16 = mybir.dt.float16
I16 = mybir.dt.int16
Identity = mybir.ActivationFunctionType.Identity
Exp = mybir.ActivationFunctionType.Exp
ts = bass.ts


@with_exitstack
def _emit(ctx: ExitStack, tc: tile.TileContext, t: dict, S: int):
    nc = tc.nc
    NQB = S // QB
    NKB = S // KBK

    wt = ctx.enter_context(tc.tile_pool(name="wt", bufs=1))
    sb = ctx.enter_context(tc.tile_pool(name="sb", bufs=1))
    wexp = ctx.enter_context(tc.tile_pool(name="wexp", bufs=4))
    nrm = ctx.enter_context(tc.tile_pool(name="nrm", bufs=2))
    ps = ctx.enter_context(tc.tile_pool(name="ps", bufs=2, space="PSUM"))
    avps = ctx.enter_context(tc.tile_pool(name="avps", bufs=4, space="PSUM"))

    # ---- persistent activations ----
    qT = sb.tile([128, S], F16)          # [proj_col, q] (sqrt(a)-scaled)
    kT = sb.tile([128, S], F16)          # [proj_col, k] (sqrt(a)-scaled)
    vaug = sb.tile([128, HPC, NKB, 33], F16)  # [k_in_blk, head, k_blk, dh+1]
    oT4 = sb.tile([32, HPC, S], F16)     # per-head attn out, rows 0-31
    m_sb = sb.tile([128, 2, NKB, QB], F16)    # mask prefetch, dbl-buffered

    # ---- constants ----
    wq_s = wt.tile([128, 2, PC], F16)
    wk_s = wt.tile([128, 2, PC], F16)
    wv_s = wt.tile([128, 2, PC], F16)
    for c in range(2):
        nc.sync.dma_start(out=wq_s[:, c, :], in_=t["wq"][ts(c, 128), :])
        nc.sync.dma_start(out=wk_s[:, c, :], in_=t["wk"][ts(c, 128), :])
        nc.sync.dma_start(out=wv_s[:, c, :], in_=t["wv"][ts(c, 128), :])
    bq_s = wt.tile([128, 1], F32)
    bk_s = wt.tile([128, 1], F32)
    nc.sync.dma_start(out=bq_s[:], in_=t["bq"][:, :])
    nc.sync.dma_start(out=bk_s[:], in_=t["bk"][:, :])
    bv_bc = wt.tile([128, PC], F32)
    nc.sync.dma_start(out=bv_bc[:], in_=t["bv"].to_broadcast([128, PC]))
    ones_sb = wt.tile([128, 32], F16)
    nc.sync.dma_start(out=ones_sb[:], in_=t["ones32"][:, :])
    wo4_s = wt.tile([32, HPC, D], F16)
    nc.sync.dma_start(out=wo4_s[:], in_=t["wo4"][:, :, :])
    nc.gpsimd.memset(vaug[:, :, :, 32:33], 1.0)

    # prefetch mask for j=0 right away
    nc.sync.dma_start(
        out=m_sb[:, 0, :, :],
        in_=t["m01"][:, ts(0, QB)].rearrange("(kb p) q -> p kb q", p=128),
    )

    with tc.tile_pool(name="xin", bufs=1) as xin:
        xq_s = xin.tile([128, 2, S], F16)
        xk_s = xin.tile([128, 2, S], F16)
        xv_s = xin.tile([128, 2, S], F16)
        for c in range(2):
            nc.sync.dma_start(out=xq_s[:, c, :], in_=t["xq"][ts(c, 128), :])
            nc.sync.dma_start(out=xk_s[:, c, :], in_=t["xk"][ts(c, 128), :])
            nc.sync.dma_start(out=xv_s[:, c, :], in_=t["xv"][ts(c, 128), :])

        # ---- q/k projections: psum = W.T @ xT  -> [proj, S] ----
        for di, (xs, ws, bs) in enumerate(
                ((xq_s, wq_s, bq_s), (xk_s, wk_s, bk_s))):
            dst16 = (qT, kT)[di]
            for j in range(NQB):
                p = ps.tile([128, 2, QB], F32, tag="mm")
                for c in range(2):
                    nc.tensor.matmul(
                        p[:, 0, :],
                        lhsT=ws[:, c, :],
                        rhs=xs[:, c, ts(j, QB)],
                        start=(c == 0),
                        stop=(c == 1),
                    )
                nc.scalar.activation(
                    out=dst16[:, ts(j, QB)], in_=p[:, 0, :],
                    func=Identity, bias=bs[:, 0:1], scale=1.0,
                )

        # ---- v projection in natural layout ----
        for sbk in range(NKB):
            p = ps.tile([128, 2, QB], F32, tag="mm")
            for c in range(2):
                nc.tensor.matmul(
                    p[:, 0, 0:PC],
                    lhsT=xv_s[:, c, ts(sbk, 128)],
                    rhs=wv_s[:, c, :],
                    start=(c == 0),
                    stop=(c == 1),
                )
            nc.vector.tensor_add(
                vaug[:, :, sbk, 0:32],
                p[:, 0, 0:PC].rearrange("p (h d) -> p h d", h=HPC),
                bv_bc[:, :].rearrange("p (h d) -> p h d", h=HPC),
            )

    # ---- attention main loop ----
    gp_idx = 0
    for j in range(NQB):
        jb = j % 2
        if j + 1 < NQB:
            nc.sync.dma_start(
                out=m_sb[:, (j + 1) % 2, :, :],
                in_=t["m01"][:, ts(j + 1, QB)]
                    .rearrange("(kb p) q -> p kb q", p=128),
            )
        av = [avps.tile([128, QB], F32, tag="av", name=f"av{h}")
              for h in range(HPC)]
        tiles = [(kb, pair) for kb in range(NKB) for pair in range(2)]
        pend = []

        def emit_front(kb, pair):
            nonlocal gp_idx
            schr = (kb % SCHR_DEN) < SCHR_NUM
            sc = ps.tile([128, 2, QB], F32, tag="mm")
            for i in range(2):
                h = 2 * pair + i
                nc.tensor.matmul(
                    sc[:, i, :],
                    lhsT=kT[32 * h:32 * h + 32, ts(kb, KBK)],
                    rhs=qT[32 * h:32 * h + 32, ts(j, QB)],
                    start=True, stop=True,
                    tile_position=(32 * h, 0),
                )
            if schr:
                wi = wexp.tile([128, 2, QB], I16, tag="w")
                nc.vector.scalar_tensor_tensor(
                    out=wi[:], in0=sc[:], scalar=SCHRB,
                    in1=m_sb[:, jb, kb, :]
                        .rearrange("p (o n) -> p o n", o=1)
                        .to_broadcast([128, 2, QB]),
                    op0=mybir.AluOpType.add, op1=mybir.AluOpType.mult,
                )
                return wi[:].bitcast(F16)
            w = wexp.tile([128, 2, QB], F16, tag="w")
            nc.scalar.activation(out=w[:], in_=sc[:], func=Exp,
                                 scale=float(1.0 / EXPA))
            wmt = wexp.tile([128, 2, QB], F16, tag="wm")
            for i in range(2):
                eng = (nc.gpsimd if (gp_idx % GP_DEN) < GP_NUM
                       else nc.vector)
                gp_idx += 1
                eng.tensor_mul(wmt[:, i, :], w[:, i, :],
                               m_sb[:, jb, kb, :])
            return wmt[:]

        def emit_av(kb, pair, wm):
            for i in range(2):
                h = 2 * pair + i
                nc.tensor.matmul(
                    av[h][0:33, :],
                    lhsT=vaug[:, h, kb, :],
                    rhs=wm[:, i, :],
                    start=(kb == 0),
                    stop=(kb == NKB - 1),
                )

        for ti, (kb, pair) in enumerate(tiles):
            wm = emit_front(kb, pair)
            pend.append((kb, pair, wm))
            if ti >= LAG:
                emit_av(*pend.pop(0))
        while pend:
            emit_av(*pend.pop(0))

        # ---- normalize: oT4 rows = av rows * (1/den) ----
        for pair in range(2):
            den = nrm.tile([128, 2, QB], F16, tag="den")
            for i in range(2):
                h = 2 * pair + i
                nc.vector.tensor_copy(out=den[32:33, i, :],
                                      in_=av[h][32:33, :])
            pb = ps.tile([128, 2, QB], F32, tag="mm")
            for i in range(2):
                nc.tensor.matmul(
                    pb[0:32, i, :], lhsT=ones_sb[32:33, :],
                    rhs=den[32:33, i, :],
                    start=True, stop=True, tile_position=(32, 0),
                )
            rec = nrm.tile([32, 2, QB], F32, tag="rec")
            nc.vector.reciprocal_approx_fast(rec[:], pb[0:32, :, :])
            for i in range(2):
                h = 2 * pair + i
                nc.vector.tensor_mul(
                    oT4[0:32, h, ts(j, QB)], av[h][0:32, :], rec[:, i, :],
                )

        # ---- output projection for this j block ----
        for qq in range(QB // 128):
            qb = j * (QB // 128) + qq
            p = ps.tile([128, 2, QB], F32, tag="mm")
            for h in range(HPC):
                nc.tensor.matmul(
                    p[:, 0, 0:D],
                    lhsT=oT4[0:32, h, ts(qb, 128)],
                    rhs=wo4_s[:, h, :],
                    start=(h == 0), stop=(h == HPC - 1),
                )
            ob = wexp.tile([128, D], F32, tag="outbuf")
            nc.vector.tensor_copy(out=ob[:], in_=p[:, 0, 0:D])
            nc.sync.dma_start(out=t["out"][ts(qb, 128), :], in_=ob[:])


def build(S: int = 2048):
    nc = bacc.Bacc("TRN2", target_bir_lowering=False, debug=False,
                   num_devices=NCORES)
    t = {}
    t["xq"] = nc.dram_tensor("xq", [D, S], F16, kind="ExternalInput").ap()
    t["xk"] = nc.dram_tensor("xk", [D, S], F16, kind="ExternalInput").ap()
    t["xv"] = nc.dram_tensor("xv", [D, S], F16, kind="ExternalInput").ap()
    t["wq"] = nc.dram_tensor("wq", [D, PC], F16, kind="ExternalInput").ap()
    t["wk"] = nc.dram_tensor("wk", [D, PC], F16, kind="ExternalInput").ap()
    t["wv"] = nc.dram_tensor("wv", [D, PC], F16, kind="ExternalInput").ap()
    t["wo4"] = nc.dram_tensor("wo4", [32, HPC, D], F16,
                              kind="ExternalInput").ap()
    t["ones32"] = nc.dram_tensor("ones32", [128, 32], F16,
                                 kind="ExternalInput").ap()
    t["bq"] = nc.dram_tensor("bq", [PC, 1], F32, kind="ExternalInput").ap()
    t["bk"] = nc.dram_tensor("bk", [PC, 1], F32, kind="ExternalInput").ap()
    t["bv"] = nc.dram_tensor("bv", [1, PC], F32, kind="ExternalInput").ap()
    t["m01"] = nc.dram_tensor("m01", [S, S], F16, kind="ExternalInput").ap()
    t["out"] = nc.dram_tensor("out", [S, D], F32, kind="ExternalOutput").ap()

    with tile.TileContext(nc) as tc:
        _emit(tc, t, S)
    nc.compile()
    return nc


_NC_CACHE = {}


def _get_nc(S):
    if S not in _NC_CACHE:
        _NC_CACHE[S] = build(S)
    return _NC_CACHE[S]


def _pack_wo4(wo_slice):
    """[PC, D] -> [32, HPC, D] per-head rows."""
    w = np.zeros((32, HPC, D), np.float32)
    for h in range(HPC):
        w[:, h, :] = wo_slice[32 * h:32 * h + 32, :]
    return w


def make_in_maps(queries, keys, values, mask, Wq, bq, Wk, bk, Wv, bv, Wo, bo):
    queries = np.asarray(queries, np.float32)
    keys = np.asarray(keys, np.float32)
    values = np.asarray(values, np.float32)
    mask = np.asarray(mask)
    Wq, Wk, Wv, Wo = (np.asarray(a, np.float32) for a in (Wq, Wk, Wv, Wo))
    bq, bk, bv, bo = (np.asarray(a, np.float32) for a in (bq, bk, bv, bo))
    S = queries.shape[1]
    sc = np.float32(1.0) / np.sqrt(np.float32(PROJ))
    ra = np.float32(np.sqrt(EXPA))
    f16 = np.float16
    in_maps = []
    for c in range(NCORES):
        b = c // 2
        p0 = PC * (c % 2)
        m01 = mask[b, 0].T.astype(f16)
        im = {
            "xq": np.ascontiguousarray(queries[b].T).astype(f16),
            "xk": np.ascontiguousarray(keys[b].T).astype(f16),
            "xv": np.ascontiguousarray(values[b].T).astype(f16),
            "wq": (Wq[:, p0:p0 + PC] * (sc * ra)).astype(f16),
            "wk": (Wk[:, p0:p0 + PC] * ra).astype(f16),
            "wv": Wv[:, p0:p0 + PC].astype(f16),
            "bq": np.ascontiguousarray(
                (bq[p0:p0 + PC] * sc * ra).reshape(PC, 1)),
            "bk": np.ascontiguousarray(
                (bk[p0:p0 + PC] * ra).reshape(PC, 1)),
            "bv": np.ascontiguousarray(bv[p0:p0 + PC].reshape(1, PC)),
            "m01": m01,
            "wo4": _pack_wo4(Wo[p0:p0 + PC, :]).astype(f16),
            "ones32": np.ones((128, 32), f16),
        }
        in_maps.append(im)
    return in_maps


def run(inputs, trace=False):
    S = np.asarray(inputs["queries"]).shape[1]
    nc = _get_nc(S)
    in_maps = make_in_maps(**inputs)
    res = run_bass_kernel_spmd(nc, in_maps, core_ids=list(range(NCORES)),
                               trace=trace)
    parts = [np.asarray(r["out"], np.float32) for r in res.results]
    bo = np.asarray(inputs["bo"], np.float32)
    out = np.zeros((B, S, D), np.float32)
    for b in range(B):
        out[b] = parts[2 * b] + parts[2 * b + 1] + bo[None, :]
    return out, res


def kernel(**inputs) -> np.ndarray:
    out, _ = run(inputs, trace=False)
    return out


# revision 19
# speedup vs baseline: 1.0841x; 1.0841x over previous
"""Multi-head attention kernel for Trainium2 (8 NeuronCores, SPMD).

Sharding: core c handles batch b=c//2 and 4 of the 8 heads
(projection columns 128*(c%2) .. +128).  Each core computes a partial
output projection; the host sums the two partials per batch and adds bo.

v2 structure: the mask is applied multiplicatively AFTER the exp
(w = exp(s) * m, m in {0,1} fp16) so the PE never runs identity-inject
matmuls and the DVE never does fp32 PSUM bias-adds.  The mask tile is
loaded once per (j, kb) and shared by all 4 heads (one 2MB prefetch per
j block).

Per core, S=2048, D=256, 4 heads of dh=32, all hot matmuls fp16:
  qT/kT = (x @ W).T in [proj, S] layout (PE; Act adds bias, fp16 out)
  v     = x @ Wv natural [S, proj] (+ ones col for fused denominator)
  sT[k,q] = sum_d kT[d,k] qT[d,q]  (4 heads row-tiled on the PE)
  w  = exp(sT) on Act (fp16), wm = w * m on DVE (2-byte mode)
  av[d,q] + den[q] = [v|1].T @ wm  (fp32 PSUM accum over kb)
  o  = av * (1/den)  (ones-matmul bcast + reciprocal + mul)
  out_partial[q,:] = sum_h o_h.T @ Wo_h
"""

import numpy as np
import ml_dtypes

import concourse.bass as bass
import concourse.tile as tile
from concourse import bacc, mybir
from concourse.bass_utils import run_bass_kernel_spmd
from concourse._compat import with_exitstack
from contextlib import ExitStack

B, D = 4, 256
H = 8
PROJ = 256
DH = PROJ // H            # 32
NCORES = 8
HPC = H // 2              # heads per core
PC = HPC * DH             # projection cols per core = 128
QB = 512                  # q block (PE moving dim / PSUM bank)
KBK = 128                 # k block

F32 = mybir.dt.float32
F16 = mybir.dt.float16
Identity = mybir.ActivationFunctionType.Identity
Exp = mybir.ActivationFunctionType.Exp
ts = bass.ts


@with_exitstack
def _emit(ctx: ExitStack, tc: tile.TileContext, t: dict, S: int):
    nc = tc.nc
    NQB = S // QB
    NKB = S // KBK

    wt = ctx.enter_context(tc.tile_pool(name="wt", bufs=1))
    sb = ctx.enter_context(tc.tile_pool(name="sb", bufs=1))
    wexp = ctx.enter_context(tc.tile_pool(name="wexp", bufs=3))
    nrm = ctx.enter_context(tc.tile_pool(name="nrm", bufs=2))
    ps = ctx.enter_context(tc.tile_pool(name="ps", bufs=2, space="PSUM"))
    avps = ctx.enter_context(tc.tile_pool(name="avps", bufs=4, space="PSUM"))

    # ---- persistent activations ----
    qT = sb.tile([128, S], F16)          # [proj_col, q]
    kT = sb.tile([128, S], F16)          # [proj_col, k]
    vaug = sb.tile([128, HPC, NKB, 33], F16)  # [k_in_blk, head, k_blk, dh+1]
    oT4 = sb.tile([32, HPC, S], F16)     # per-head attn out, rows 0-31
    m_sb = sb.tile([128, 2, NKB, QB], F16)    # mask prefetch, dbl-buffered

    # ---- constants ----
    wq_s = wt.tile([128, 2, PC], F16)
    wk_s = wt.tile([128, 2, PC], F16)
    wv_s = wt.tile([128, 2, PC], F16)
    for c in range(2):
        nc.sync.dma_start(out=wq_s[:, c, :], in_=t["wq"][ts(c, 128), :])
        nc.sync.dma_start(out=wk_s[:, c, :], in_=t["wk"][ts(c, 128), :])
        nc.sync.dma_start(out=wv_s[:, c, :], in_=t["wv"][ts(c, 128), :])
    bq_s = wt.tile([128, 1], F32)
    bk_s = wt.tile([128, 1], F32)
    nc.sync.dma_start(out=bq_s[:], in_=t["bq"][:, :])
    nc.sync.dma_start(out=bk_s[:], in_=t["bk"][:, :])
    bv_bc = wt.tile([128, PC], F32)
    nc.sync.dma_start(out=bv_bc[:], in_=t["bv"].to_broadcast([128, PC]))
    ones_sb = wt.tile([128, 32], F16)
    nc.sync.dma_start(out=ones_sb[:], in_=t["ones32"][:, :])
    wo4_s = wt.tile([32, HPC, D], F16)
    nc.sync.dma_start(out=wo4_s[:], in_=t["wo4"][:, :, :])
    nc.gpsimd.memset(vaug[:, :, :, 32:33], 1.0)

    # prefetch mask for j=0 right away
    nc.sync.dma_start(
        out=m_sb[:, 0, :, :],
        in_=t["m01"][:, ts(0, QB)].rearrange("(kb p) q -> p kb q", p=128),
    )

    with tc.tile_pool(name="xin", bufs=1) as xin:
        xq_s = xin.tile([128, 2, S], F16)
        xk_s = xin.tile([128, 2, S], F16)
        xv_s = xin.tile([128, 2, S], F16)
        for c in range(2):
            nc.sync.dma_start(out=xq_s[:, c, :], in_=t["xq"][ts(c, 128), :])
            nc.sync.dma_start(out=xk_s[:, c, :], in_=t["xk"][ts(c, 128), :])
            nc.sync.dma_start(out=xv_s[:, c, :], in_=t["xv"][ts(c, 128), :])

        # ---- q/k projections: psum = W.T @ xT  -> [proj, S] ----
        for dst, xs, ws, bs in ((qT, xq_s, wq_s, bq_s), (kT, xk_s, wk_s, bk_s)):
            for j in range(NQB):
                p = ps.tile([128, 2, QB], F32, tag="mm")
                for c in range(2):
                    nc.tensor.matmul(
                        p[:, 0, :],
                        lhsT=ws[:, c, :],
                        rhs=xs[:, c, ts(j, QB)],
                        start=(c == 0),
                        stop=(c == 1),
                    )
                nc.scalar.activation(
                    out=dst[:, ts(j, QB)], in_=p[:, 0, :],
                    func=Identity, bias=bs[:, 0:1], scale=1.0,
                )

        # ---- v projection in natural layout ----
        for sbk in range(NKB):
            p = ps.tile([128, 2, QB], F32, tag="mm")
            for c in range(2):
                nc.tensor.matmul(
                    p[:, 0, 0:PC],
                    lhsT=xv_s[:, c, ts(sbk, 128)],
                    rhs=wv_s[:, c, :],
                    start=(c == 0),
                    stop=(c == 1),
                )
            nc.vector.tensor_add(
                vaug[:, :, sbk, 0:32],
                p[:, 0, 0:PC].rearrange("p (h d) -> p h d", h=HPC),
                bv_bc[:, :].rearrange("p (h d) -> p h d", h=HPC),
            )

    # ---- attention main loop ----
    for j in range(NQB):
        jb = j % 2
        if j + 1 < NQB:
            nc.sync.dma_start(
                out=m_sb[:, (j + 1) % 2, :, :],
                in_=t["m01"][:, ts(j + 1, QB)]
                    .rearrange("(kb p) q -> p kb q", p=128),
            )
        av = [avps.tile([128, QB], F32, tag="av", name=f"av{h}")
              for h in range(HPC)]
        for kb in range(NKB):
            for pair in range(2):
                sc = ps.tile([128, 2, QB], F32, tag="mm")
                for i in range(2):
                    h = 2 * pair + i
                    nc.tensor.matmul(
                        sc[:, i, :],
                        lhsT=kT[32 * h:32 * h + 32, ts(kb, KBK)],
                        rhs=qT[32 * h:32 * h + 32, ts(j, QB)],
                        start=True, stop=True,
                        tile_position=(32 * h, 0),
                    )
                w = wexp.tile([128, 2, QB], F16, tag="w")
                nc.scalar.activation(out=w[:], in_=sc[:], func=Exp)
                wm = wexp.tile([128, 2, QB], F16, tag="wm")
                nc.vector.tensor_mul(
                    wm[:],
                    w[:],
                    m_sb[:, jb, kb, :]
                        .rearrange("p (o n) -> p o n", o=1)
                        .to_broadcast([128, 2, QB]),
                )
                for i in range(2):
                    h = 2 * pair + i
                    nc.tensor.matmul(
                        av[h][0:33, :],
                        lhsT=vaug[:, h, kb, :],
                        rhs=wm[:, i, :],
                        start=(kb == 0),
                        stop=(kb == NKB - 1),
                    )
        # ---- normalize: oT4 rows = av rows * (1/den) ----
        for pair in range(2):
            den = nrm.tile([128, 2, QB], F16, tag="den")
            for i in range(2):
                h = 2 * pair + i
                nc.vector.tensor_copy(out=den[32:33, i, :],
                                      in_=av[h][32:33, :])
            pb = ps.tile([128, 2, QB], F32, tag="mm")
            for i in range(2):
                nc.tensor.matmul(
                    pb[0:32, i, :], lhsT=ones_sb[32:33, :],
                    rhs=den[32:33, i, :],
                    start=True, stop=True, tile_position=(32, 0),
                )
            rec = nrm.tile([32, 2, QB], F32, tag="rec")
            nc.vector.reciprocal_approx_fast(rec[:], pb[0:32, :, :])
            for i in range(2):
                h = 2 * pair + i
                nc.vector.tensor_mul(
                    oT4[0:32, h, ts(j, QB)], av[h][0:32, :], rec[:, i, :],
                )

    # ---- output projection: out[q, :] = sum_h oT_h.T @ Wo_h ----
    for qb in range(S // 128):
        p = ps.tile([128, 2, QB], F32, tag="mm")
        for h in range(HPC):
            nc.tensor.matmul(
                p[:, 0, 0:D],
                lhsT=oT4[0:32, h, ts(qb, 128)],
                rhs=wo4_s[:, h, :],
                start=(h == 0), stop=(h == HPC - 1),
            )
        ob = wexp.tile([128, D], F32, tag="outbuf")
        nc.vector.tensor_copy(out=ob[:], in_=p[:, 0, 0:D])
        nc.sync.dma_start(out=t["out"][ts(qb, 128), :], in_=ob[:])


def build(S: int = 2048):
    nc = bacc.Bacc("TRN2", target_bir_lowering=False, debug=False,
                   num_devices=NCORES)
    t = {}
    t["xq"] = nc.dram_tensor("xq", [D, S], F16, kind="ExternalInput").ap()
    t["xk"] = nc.dram_tensor("xk", [D, S], F16, kind="ExternalInput").ap()
    t["xv"] = nc.dram_tensor("xv", [D, S], F16, kind="ExternalInput").ap()
    t["wq"] = nc.dram_tensor("wq", [D, PC], F16, kind="ExternalInput").ap()
    t["wk"] = nc.dram_tensor("wk", [D, PC], F16, kind="ExternalInput").ap()
    t["wv"] = nc.dram_tensor("wv", [D, PC], F16, kind="ExternalInput").ap()
    t["wo4"] = nc.dram_tensor("wo4", [32, HPC, D], F16,
                              kind="ExternalInput").ap()
    t["ones32"] = nc.dram_tensor("ones32", [128, 32], F16,
                                 kind="ExternalInput").ap()
    t["bq"] = nc.dram_tensor("bq", [PC, 1], F32, kind="ExternalInput").ap()
    t["bk"] = nc.dram_tensor("bk", [PC, 1], F32, kind="ExternalInput").ap()
    t["bv"] = nc.dram_tensor("bv", [1, PC], F32, kind="ExternalInput").ap()
    t["m01"] = nc.dram_tensor("m01", [S, S], F16, kind="ExternalInput").ap()
    t["out"] = nc.dram_tensor("out", [S, D], F32, kind="ExternalOutput").ap()

    with tile.TileContext(nc) as tc:
        _emit(tc, t, S)
    nc.compile()
    return nc


_NC_CACHE = {}


def _get_nc(S):
    if S not in _NC_CACHE:
        _NC_CACHE[S] = build(S)
    return _NC_CACHE[S]


def _pack_wo4(wo_slice):
    """[PC, D] -> [32, HPC, D] per-head rows."""
    w = np.zeros((32, HPC, D), np.float32)
    for h in range(HPC):
        w[:, h, :] = wo_slice[32 * h:32 * h + 32, :]
    return w


def make_in_maps(queries, keys, values, mask, Wq, bq, Wk, bk, Wv, bv, Wo, bo):
    queries = np.asarray(queries, np.float32)
    keys = np.asarray(keys, np.float32)
    values = np.asarray(values, np.float32)
    mask = np.asarray(mask)
    Wq, Wk, Wv, Wo = (np.asarray(a, np.float32) for a in (Wq, Wk, Wv, Wo))
    bq, bk, bv, bo = (np.asarray(a, np.float32) for a in (bq, bk, bv, bo))
    S = queries.shape[1]
    sc = np.float32(1.0) / np.sqrt(np.float32(PROJ))
    f16 = np.float16
    in_maps = []
    for c in range(NCORES):
        b = c // 2
        p0 = PC * (c % 2)
        m01 = mask[b, 0].T.astype(f16)
        im = {
            "xq": np.ascontiguousarray(queries[b].T).astype(f16),
            "xk": np.ascontiguousarray(keys[b].T).astype(f16),
            "xv": np.ascontiguousarray(values[b].T).astype(f16),
            "wq": (Wq[:, p0:p0 + PC] * sc).astype(f16),
            "wk": Wk[:, p0:p0 + PC].astype(f16),
            "wv": Wv[:, p0:p0 + PC].astype(f16),
            "bq": np.ascontiguousarray((bq[p0:p0 + PC] * sc).reshape(PC, 1)),
            "bk": np.ascontiguousarray(bk[p0:p0 + PC].reshape(PC, 1)),
            "bv": np.ascontiguousarray(bv[p0:p0 + PC].reshape(1, PC)),
            "m01": m01,
            "wo4": _pack_wo4(Wo[p0:p0 + PC, :]).astype(f16),
            "ones32": np.ones((128, 32), f16),
        }
        in_maps.append(im)
    return in_maps


def run(inputs, trace=False):
    S = np.asarray(inputs["queries"]).shape[1]
    nc = _get_nc(S)
    in_maps = make_in_maps(**inputs)
    res = run_bass_kernel_spmd(nc, in_maps, core_ids=list(range(NCORES)),
                               trace=trace)
    parts = [np.asarray(r["out"], np.float32) for r in res.results]
    bo = np.asarray(inputs["bo"], np.float32)
    out = np.zeros((B, S, D), np.float32)
    for b in range(B):
        out[b] = parts[2 * b] + parts[2 * b + 1] + bo[None, :]
    return out, res


def kernel(**inputs) -> np.ndarray:
    out, _ = run(inputs, trace=False)
    return out
